# revision 12
# baseline (speedup 1.0000x reference)
"""GATv2 (2-layer, heads=1) on 8 Trainium2 NeuronCores via Bass/Tile.

Sharding: nodes (and their incoming edges) are partitioned across the 8
cores by destination node. Each core projects its local nodes (xl = x@Wl,
xr = x@Wr), AllGathers the source-side projection into a per-core HBM
table, then processes its edges in dst-blocks of 128 nodes:
  - dma_gather of xl[src] and xr[dst] per edge (edge-major [128, T, 256])
  - z = xl[src] + xr[dst] (one DVE add per block)
  - GATv2 score via ACT Prelu(alpha=0.2) + accum_out. |att_c| is folded
    into the projection weight columns on the host (channels reordered so
    negative-att channels come first), so
      score = sum_pos prelu(z~) - sum_neg prelu(z~).
  - softmax numerator exp(score) is folded into a one-hot matrix
    W[e, n] = (iota == dstoff_e) * exp(score_e) built by one tensor_scalar
    op; PE matmuls accumulate both sum_e es*z~ and sum_e es (denominator)
    in PSUM across the block's tiles. Normalization, |att|-descale, bias
    and relu happen once per block on DVE.
"""

import os
import numpy as np

import concourse.bacc as bacc
import concourse.tile as tile
import concourse.mybir as mybir
from concourse import library_config
from concourse.bass_utils import run_bass_kernel_spmd

N = 10000
E = 320000
D = 256
DOUT = 64
NEG = 0.2
NCORES = 8
NLOC = N // NCORES          # 1250 dst nodes per core
NBLK = (NLOC + 127) // 128  # 10 dst blocks per core
FP = mybir.dt.float32
I16 = mybir.dt.int16

LAST_RESULT = None  # BassKernelResults of the most recent run (for test.py)

_cache = {}  # (schedule, k1, k2) -> compiled nc


def _pack_idx(flat):
    """Pack a flat int index list (len % 128 == 0) into the SBUF layout
    dma_gather expects: [16, n/16] column-wrapped, tiled to 128 partitions."""
    n = len(flat)
    a = flat.reshape(n // 128, 8, 16).transpose(2, 0, 1).reshape(16, n // 16)
    return np.tile(a, (8, 1))


def _host_prep(inputs):
    ei = np.asarray(inputs["edge_index"]).astype(np.int64)
    src = np.concatenate([ei[0], np.arange(N, dtype=np.int64)])
    dst = np.concatenate([ei[1], np.arange(N, dtype=np.int64)])
    order = np.argsort(dst, kind="stable")
    src, dst = src[order], dst[order]

    # per (core, block) edge slices
    cnt = np.zeros((NCORES, NBLK), dtype=np.int64)
    lo = np.zeros((NCORES, NBLK), dtype=np.int64)
    hi = np.zeros((NCORES, NBLK), dtype=np.int64)
    for k in range(NCORES):
        for b in range(NBLK):
            nlo = k * NLOC + b * 128
            nhi = min(k * NLOC + (b + 1) * 128, (k + 1) * NLOC)
            lo[k][b] = np.searchsorted(dst, nlo)
            hi[k][b] = np.searchsorted(dst, nhi)
            cnt[k][b] = hi[k][b] - lo[k][b]
    TBL = [int(max(1, np.ceil(cnt[:, b].max() / 128))) for b in range(NBLK)]

    per_core = []
    for k in range(NCORES):
        s16, d16, doff = [], [], []
        for b in range(NBLK):
            l, h = lo[k][b], hi[k][b]
            npad = TBL[b] * 128
            sb = np.zeros(npad, dtype=np.int64)
            sb[: h - l] = src[l:h]
            db = np.zeros(npad, dtype=np.int64)
            db[: h - l] = dst[l:h] - k * NLOC
            ob = np.full(npad, -1.0, dtype=np.float32)
            ob[: h - l] = (dst[l:h] - (k * NLOC + b * 128)).astype(np.float32)
            s16.append(_pack_idx(sb.astype(np.int16)))
            d16.append(_pack_idx(db.astype(np.int16)))
            doff.append(ob.reshape(TBL[b], 128).T)  # [128, Tb]
        per_core.append(
            dict(
                src16=np.concatenate(s16, axis=1),
                dst16=np.concatenate(d16, axis=1),
                dstoff=np.concatenate(doff, axis=1),
            )
        )
    return TBL, per_core


def _fold_att(att):
    """Channel permutation (negative att first), |att| scales, inverse."""
    att = np.asarray(att, dtype=np.float64)
    neg = np.where(att < 0)[0]
    pos = np.where(att >= 0)[0]
    perm = np.concatenate([neg, pos])
    k = len(neg)
    s = np.abs(att[perm])
    s = np.where(s == 0, 1e-30, s)
    sinv = 1.0 / s
    return perm, k, s.astype(np.float32), sinv.astype(np.float32)


def _wdev(w):
    """[256, C] weight -> [128, 2*C] (two contraction halves side by side)."""
    return np.concatenate([w[:128], w[128:]], axis=1).astype(np.float32)


def _bcast(v):
    return np.tile(np.asarray(v, dtype=np.float32)[None, :], (128, 1))


def _build(TBL, k1, k2):
    TT = sum(TBL)
    nc = bacc.Bacc("TRN2", target_bir_lowering=False, debug=False,
                   num_devices=NCORES)

    def inp(name, shape, dtype=FP):
        return nc.dram_tensor(name, shape, dtype, kind="ExternalInput")

    xT = inp("xT", [128, 2 * NBLK * 128])
    w1l = inp("w1l", [128, 2 * D]); w1r = inp("w1r", [128, 2 * D])
    w2l = inp("w2l", [128, 2 * D]); w2r = inp("w2r", [128, 2 * D])
    wout = inp("wout", [128, 2 * DOUT])
    b1l = inp("b1l", [128, D]); b1r = inp("b1r", [128, D])
    b2l = inp("b2l", [128, D]); b2r = inp("b2r", [128, D])
    bias1 = inp("bias1", [128, D]); bias2 = inp("bias2", [128, D])
    boutb = inp("boutb", [128, DOUT])
    sinv1 = inp("sinv1", [128, D]); sinv2 = inp("sinv2", [128, D])
    iota = inp("iota", [128, 128])
    ident = inp("ident", [128, 128])
    ones = inp("ones", [128, 1])
    src16 = inp("src16", [128, 8 * TT], I16)
    dst16 = inp("dst16", [128, 8 * TT], I16)
    dstoff = inp("dstoff", [128, TT])
    out = nc.dram_tensor("out", [NLOC, DOUT], FP, kind="ExternalOutput")

    with tile.TileContext(nc) as tc:
        with tc.tile_critical():
            nc.gpsimd.load_library(library_config.mlp)
        with (
            tc.tile_pool(name="cst", bufs=1) as cst,
            tc.tile_pool(name="sb", bufs=1) as sb,
            tc.tile_pool(name="sbw", bufs=3) as sbw,
            tc.tile_pool(name="ps", bufs=2, space="PSUM") as psp,
            tc.tile_pool(name="ps1", bufs=1, space="PSUM") as psp1,
            tc.tile_pool(name="dram", bufs=1, space="DRAM") as dram,
        ):
            # ---- load constants / inputs into SBUF
            def load(src_ap, shape, dtype=FP):
                t = cst.tile(shape, dtype, tag=src_ap.name)
                nc.sync.dma_start(t[:], src_ap[:])
                return t

            xT_sb = load(xT, [128, 2 * NBLK * 128])
            w_sb = {1: (load(w1l, [128, 2 * D]), load(w1r, [128, 2 * D])),
                    2: (load(w2l, [128, 2 * D]), load(w2r, [128, 2 * D]))}
            wb_sb = {1: (load(b1l, [128, D]), load(b1r, [128, D])),
                     2: (load(b2l, [128, D]), load(b2r, [128, D]))}
            bias_sb = {1: load(bias1, [128, D]), 2: load(bias2, [128, D])}
            sinv_sb = {1: load(sinv1, [128, D]), 2: load(sinv2, [128, D])}
            wout_sb = load(wout, [128, 2 * DOUT])
            bout_sb = load(boutb, [128, DOUT])
            iota_sb = load(iota, [128, 128])
            id_sb = load(ident, [128, 128])
            ones_sb = load(ones, [128, 1])
            src_sb = load(src16, [128, 8 * TT], I16)
            dst_sb = load(dst16, [128, 8 * TT], I16)
            doff_sb = load(dstoff, [128, TT])

            # ---- DRAM tables
            xl_slab = dram.tile([NLOC, D], FP)
            xl_table = dram.tile([N, D], FP)
            xr_slab = dram.tile([NLOC, D], FP)

            # ---- persistent SBUF activations
            xr_all = sb.tile([128, NBLK, D], FP, tag="xr_all")
            h_all = {}

            ksplit = {1: k1, 2: k2}

            def node_phase(layer, featT):
                wl, wr = w_sb[layer]
                bl, br = wb_sb[layer]
                for nb in range(NBLK):
                    rows = min(128, NLOC - nb * 128)
                    for which, wmat, bmat in (("l", wl, bl), ("r", wr, br)):
                        pnode = psp.tile([128, D], FP, tag="pnode")
                        for h in range(2):
                            nc.tensor.matmul(
                                pnode[:],
                                featT[:, h * (NBLK * 128) + nb * 128:
                                      h * (NBLK * 128) + nb * 128 + 128],
                                wmat[:, h * D:(h + 1) * D],
                                start=(h == 0), stop=(h == 1),
                            )
                        if which == "l":
                            xl_b = sbw.tile([128, D], FP, tag="xl_b")
                            nc.vector.tensor_tensor(
                                xl_b[:], pnode[:], bmat[:], mybir.AluOpType.add)
                            nc.sync.dma_start(
                                xl_slab[nb * 128: nb * 128 + rows, :],
                                xl_b[:rows, :])
                        else:
                            nc.vector.tensor_tensor(
                                xr_all[:, nb, :], pnode[:], bmat[:],
                                mybir.AluOpType.add)
                            nc.sync.dma_start(
                                xr_slab[nb * 128: nb * 128 + rows, :],
                                xr_all[:rows, nb, :])
                nc.gpsimd.collective_compute(
                    "AllGather", mybir.AluOpType.bypass,
                    replica_groups=[list(range(NCORES))],
                    ins=[xl_slab.opt()], outs=[xl_table.opt()],
                )

            edge_lvl = int(os.environ.get("GAT_EDGE", "9"))

            def edge_phase(layer):
                k = ksplit[layer]
                h_out = sb.tile([128, NBLK, D], FP, tag=f"h{layer}", name=f"h{layer}")
                h_all[layer] = h_out
                off = 0
                for b in range(NBLK):
                    Tb = TBL[b]
                    nidx = Tb * 128
                    X = sb.tile([128, Tb, D], FP, tag="X")
                    XR = sb.tile([128, Tb, D], FP, tag="XR")
                    nc.gpsimd.dma_gather(
                        X[:], xl_table[:], src_sb[:, 8 * off: 8 * (off + Tb)],
                        nidx, nidx, D, single_packet=False)
                    if edge_lvl < 2:
                        nc.vector.tensor_copy(h_out[:, b, :], X[:, 0, :])
                        off += Tb
                        continue
                    nc.gpsimd.dma_gather(
                        XR[:], xr_slab[:], dst_sb[:, 8 * off: 8 * (off + Tb)],
                        nidx, nidx, D, single_packet=False)
                    nc.vector.tensor_tensor(
                        X[:], X[:], XR[:], mybir.AluOpType.add)
                    if edge_lvl < 3:
                        nc.vector.tensor_copy(h_out[:, b, :], X[:, 0, :])
                        off += Tb
                        continue

                    scn = sbw.tile([128, Tb], FP, tag="scn")
                    scp = sbw.tile([128, Tb], FP, tag="scp")
                    scr1 = sbw.tile([128, D], FP, tag="scr1")
                    scr2 = sbw.tile([128, D], FP, tag="scr2")
                    for t in range(Tb):
                        if k > 0:
                            nc.scalar.activation(
                                scr1[:, :k], X[:, t, :k],
                                mybir.ActivationFunctionType.Prelu,
                                bias=0.0, scale=1.0, alpha=NEG,
                                accum_out=scn[:, t:t + 1])
                        if k < D:
                            nc.scalar.activation(
                                scr2[:, :D - k], X[:, t, k:],
                                mybir.ActivationFunctionType.Prelu,
                                bias=0.0, scale=1.0, alpha=NEG,
                                accum_out=scp[:, t:t + 1])
                    sc = sbw.tile([128, Tb], FP, tag="sc")
                    if k == 0:
                        nc.vector.tensor_copy(sc[:], scp[:])
                    elif k == D:
                        nc.vector.tensor_scalar_mul(sc[:], scn[:], -1.0)
                    else:
                        nc.vector.tensor_tensor(
                            sc[:], scp[:], scn[:], mybir.AluOpType.subtract)
                    es = sbw.tile([128, Tb], FP, tag="es")
                    nc.scalar.activation(
                        es[:], sc[:], mybir.ActivationFunctionType.Exp)
                    if edge_lvl < 4:
                        nc.vector.tensor_copy(h_out[:, b, :], X[:, 0, :])
                        off += Tb
                        continue

                    pagg = psp.tile([128, D], FP, tag="pagg")
                    pseg = psp1.tile([128, 1], FP, tag="pseg")
                    for t in range(Tb):
                        Wt = sbw.tile([128, 128], FP, tag="Wt")
                        nc.vector.tensor_scalar(
                            Wt[:], iota_sb[:], doff_sb[:, off + t: off + t + 1],
                            es[:, t:t + 1],
                            mybir.AluOpType.is_equal, mybir.AluOpType.mult)
                        nc.tensor.matmul(pagg[:], Wt[:], X[:, t, :],
                                         start=(t == 0), stop=(t == Tb - 1))
                        nc.tensor.matmul(pseg[:], Wt[:], ones_sb[:],
                                         start=(t == 0), stop=(t == Tb - 1))

                    rs = sbw.tile([128, 1], FP, tag="rs")
                    nc.vector.reciprocal(rs[:], pseg[:])
                    o = sbw.tile([128, D], FP, tag="o")
                    nc.vector.tensor_scalar(
                        o[:], pagg[:], rs[:], None, mybir.AluOpType.mult)
                    nc.vector.tensor_tensor(
                        o[:], o[:], xr_all[:, b, :], mybir.AluOpType.subtract)
                    nc.vector.tensor_tensor(
                        o[:], o[:], sinv_sb[layer][:], mybir.AluOpType.mult)
                    nc.vector.tensor_tensor(
                        o[:], o[:], bias_sb[layer][:], mybir.AluOpType.add)
                    nc.vector.tensor_scalar_max(h_out[:, b, :], o[:], 0.0)
                    off += Tb

            def transpose_h(layer):
                featT = sb.tile([128, 2 * NBLK * 128], FP, tag="featT")
                for nb in range(NBLK):
                    for ch in range(2):
                        tp = psp1.tile([128, 128], FP, tag="tp")
                        nc.tensor.transpose(
                            tp[:], h_all[layer][:, nb, ch * 128:(ch + 1) * 128],
                            id_sb[:])
                        nc.vector.tensor_copy(
                            featT[:, ch * (NBLK * 128) + nb * 128:
                                  ch * (NBLK * 128) + nb * 128 + 128],
                            tp[:])
                return featT

            # ---- the network
            stage = int(os.environ.get("GAT_STAGE", "9"))
            node_phase(1, xT_sb)
            if stage >= 2:
                edge_phase(1)
            else:
                h1_stub = sb.tile([128, NBLK, D], FP, tag="h1")
                h_all[1] = h1_stub
                nc.vector.tensor_copy(h_all[1][:, 0, :], xr_all[:, 0, :])
            feat2 = transpose_h(1)
            if stage >= 3:
                node_phase(2, feat2)
            if stage >= 4:
                edge_phase(2)
            else:
                h2_stub = sb.tile([128, NBLK, D], FP, tag="h2")
                h_all[2] = h2_stub
                nc.vector.tensor_copy(h_all[2][:, 0, :], xr_all[:, 0, :])
            feat3 = transpose_h(2)

            # ---- readout: logits + log_softmax
            for nb in range(NBLK):
                rows = min(128, NLOC - nb * 128)
                pl = psp1.tile([128, DOUT], FP, tag="pl")
                for h in range(2):
                    nc.tensor.matmul(
                        pl[:],
                        feat3[:, h * (NBLK * 128) + nb * 128:
                              h * (NBLK * 128) + nb * 128 + 128],
                        wout_sb[:, h * DOUT:(h + 1) * DOUT],
                        start=(h == 0), stop=(h == 1))
                lg = sbw.tile([128, DOUT], FP, tag="lg")
                nc.vector.tensor_tensor(lg[:], pl[:], bout_sb[:],
                                        mybir.AluOpType.add)
                rmax = sbw.tile([128, 1], FP, tag="rmax")
                nc.vector.tensor_reduce(rmax[:], lg[:],
                                        axis=mybir.AxisListType.XYZW,
                                        op=mybir.AluOpType.max)
                nrmax = sbw.tile([128, 1], FP, tag="nrmax")
                nc.vector.tensor_scalar_mul(nrmax[:], rmax[:], -1.0)
                ex = sbw.tile([128, DOUT], FP, tag="ex")
                nc.scalar.activation(ex[:], lg[:],
                                     mybir.ActivationFunctionType.Exp,
                                     bias=nrmax[:], scale=1.0)
                ssum = sbw.tile([128, 1], FP, tag="ssum")
                nc.vector.tensor_reduce(ssum[:], ex[:],
                                        axis=mybir.AxisListType.XYZW,
                                        op=mybir.AluOpType.add)
                lse = sbw.tile([128, 1], FP, tag="lse")
                nc.scalar.activation(lse[:], ssum[:],
                                     mybir.ActivationFunctionType.Ln)
                res = sbw.tile([128, DOUT], FP, tag="res")
                nc.vector.tensor_scalar(
                    res[:], lg[:], rmax[:], lse[:],
                    mybir.AluOpType.subtract, mybir.AluOpType.subtract)
                nc.sync.dma_start(out[nb * 128: nb * 128 + rows, :],
                                  res[:rows, :])

    nc.compile()
    return nc


def kernel(**inputs):
    global LAST_RESULT
    TBL, per_core = _host_prep(inputs)

    p1, k1, s1, sinv1 = _fold_att(inputs["att1"])
    p2, k2, s2, sinv2 = _fold_att(inputs["att2"])

    W1l = np.asarray(inputs["W1l"], np.float64)[:, p1] * s1
    W1r = np.asarray(inputs["W1r"], np.float64)[:, p1] * s1
    b1l = np.asarray(inputs["b1l"], np.float64)[p1] * s1
    b1r = np.asarray(inputs["b1r"], np.float64)[p1] * s1
    W2l = np.asarray(inputs["W2l"], np.float64)[p1][:, p2] * s2
    W2r = np.asarray(inputs["W2r"], np.float64)[p1][:, p2] * s2
    b2l = np.asarray(inputs["b2l"], np.float64)[p2] * s2
    b2r = np.asarray(inputs["b2r"], np.float64)[p2] * s2
    Wout = np.asarray(inputs["Wout"], np.float64)[p2]
    bias1 = np.asarray(inputs["bias1"], np.float64)[p1]
    bias2 = np.asarray(inputs["bias2"], np.float64)[p2]

    x = np.asarray(inputs["x"], np.float32)
    xT = x.T.astype(np.float32)  # [256, N]

    common = dict(
        w1l=_wdev(W1l), w1r=_wdev(W1r), w2l=_wdev(W2l), w2r=_wdev(W2r),
        wout=_wdev(Wout),
        b1l=_bcast(b1l), b1r=_bcast(b1r), b2l=_bcast(b2l), b2r=_bcast(b2r),
        bias1=_bcast(bias1), bias2=_bcast(bias2),
        boutb=_bcast(np.asarray(inputs["bout"], np.float64)),
        sinv1=_bcast(sinv1), sinv2=_bcast(sinv2),
        iota=np.tile(np.arange(128, dtype=np.float32), (128, 1)),
        ident=np.eye(128, dtype=np.float32),
        ones=np.ones((128, 1), np.float32),
    )

    in_maps = []
    for k in range(NCORES):
        xTk = np.zeros((D, NBLK * 128), np.float32)
        xTk[:, :NLOC] = xT[:, k * NLOC:(k + 1) * NLOC]
        xTdev = np.concatenate([xTk[:128], xTk[128:]], axis=1)
        pc = per_core[k]
        in_maps.append(dict(
            common,
            xT=xTdev,
            src16=pc["src16"], dst16=pc["dst16"], dstoff=pc["dstoff"],
        ))

    key = (tuple(TBL), k1, k2, os.environ.get("GAT_STAGE", "9"), os.environ.get("GAT_EDGE", "9"))
    if key not in _cache:
        _cache[key] = _build(TBL, k1, k2)
    nc = _cache[key]

    last_err = None
    for _ in range(3):
        try:
            res = run_bass_kernel_spmd(nc, in_maps, core_ids=list(range(NCORES)))
            break
        except Exception as e:  # axon device occasionally needs one retry
            last_err = e
    else:
        raise last_err
    LAST_RESULT = res

    outp = np.concatenate([res.results[k]["out"] for k in range(NCORES)], axis=0)
    return outp.astype(np.float32)
